# revision 6
# baseline (speedup 1.0000x reference)
"""DeTPP loss kernel for 8 TRN2 NeuronCores (batch-parallel SPMD Bass/Tile).

Strategy: shard along batch B (8 per core). Host prep does only index
plumbing on tiny tensors (row ids, the K*T picked-category logits, the
L1 window fields, the valid mask: ~0.3MB/core vs the 32MB logits table).
The memory-heavy work stays on device, per core:
  - one fp8-e4m3 logits table [R, K*C] in DRAM; the 2048 needed rows
    (1KB each, 2MB total) are fetched with chunked indirect row-gather
    DMAs (one SWDGE descriptor per row, 16 HW DMA engines in parallel),
  - ACT exp (fp8 in, bf16 out) in 2048-wide instructions; per-(n,k)
    softmax denominators via a tree of 2x-mode bf16 tensor_tensor adds
    (tensor_reduce has no DVE fast mode) + one f32 reduce; one ACT Ln
    gives the logsumexp terms of the CE cost. The fp8 rounding only
    perturbs the loss additively (the lse term is constant across the
    assignment search), and averages out over the 16k items.
  - cost entries cost[n,k,t] = |ot-dt| + |oa-a| - picked_logit assembled
    with 6 bulk DVE ops on broadcast APs while the gathers stream; the
    lse part is added after the min,
  - 24-permutation totals via PE: transpose + block-diagonal 0/1 matmul
    per half; DVE segmented min = exact Hungarian optimum for K=4,
  - masked sum and count reduced across partitions on gpsimd; host sums
    the 8 per-core (sum, count) pairs: loss = sum / (count * K).
"""
import sys

sys.path.insert(0, '/opt/trn_rl_repo')

import itertools
import numpy as np
import ml_dtypes

BF16 = ml_dtypes.bfloat16
FP8 = ml_dtypes.float8_e4m3

L, B, I, K, C = 1024, 64, 256, 4, 256
BS = B // 8            # batch per core
R = L * BS             # rows per core (8192), row id r = l*BS + b
N = I * BS             # gathered items per core (2048)
NT = N // 128          # 16 n-tiles; item n = p*NT + t  (p = partition)
PERMS = np.array(list(itertools.permutations(range(K))), dtype=np.int32)
NP_ = PERMS.shape[0]   # 24

# bigc packed-constant column layout (all bf16)
O_PICK, O_REC, O_PMAT, O_ID, O_VAL = 0, 256, 512, 704, 832
W_BIGC = 848
# rec field offsets within the 16-wide per-item rec block
F_DT, F_A, F_OT, F_OA = 0, 4, 8, 12

GATHER_CHUNKS = [(0, 1), (1, 1), (2, 2), (4, 4), (8, 8)]   # (tile start, width)
EXP_CHUNKS = [(0, 1), (1, 1)] + [(t, 2) for t in range(2, NT, 2)]


def _host_prep(core, time, amount, out_time, out_amount, out_cat_logits, cat,
               lengths, indices, consts):
    bsl = slice(core * BS, (core + 1) * BS)
    idx = indices[:, bsl].astype(np.int64)                    # (I, BS)
    bb = np.broadcast_to(np.arange(BS)[None, :], idx.shape)   # (I, BS)
    pos = (idx[:, :, None] + 1 + np.arange(K)[None, None, :]) % L  # (I,BS,K)
    bb3 = np.broadcast_to(bb[:, :, None], pos.shape)

    tloc = time[:, bsl]
    dt = tloc[pos, bb3] - tloc[idx, bb][:, :, None]           # (I, BS, K)
    aw = amount[:, bsl][pos, bb3]                             # (I, BS, K)
    cw = cat[:, bsl][pos, bb3].astype(np.int64)               # (I, BS, K)
    ot = out_time[:, bsl][idx, bb]                            # (I, BS, K)
    oa = out_amount[:, bsl][idx, bb]                          # (I, BS, K)
    ocl = out_cat_logits[:, bsl]                              # (L, BS, K, C)
    kk = np.arange(K)[None, None, :, None]
    picked = ocl[idx[:, :, None, None], bb[:, :, None, None], kk,
                 cw[:, :, None, :]]                           # (I, BS, K, T)
    valid = (idx + K < lengths[bsl].astype(np.int64)[None, :])

    bigc = np.zeros((128, W_BIGC), BF16)
    bigc[:, O_PICK:O_PICK + 256] = picked.reshape(128, 256).astype(BF16)
    rec = np.concatenate([dt, aw, ot, oa], axis=-1)           # (I, BS, 16)
    bigc[:, O_REC:O_REC + 256] = rec.reshape(128, 256).astype(BF16)
    bigc[:, O_PMAT:O_PMAT + 192] = consts["pmat"]
    bigc[:, O_ID:O_ID + 128] = consts["ident"]
    bigc[:, O_VAL:O_VAL + 16] = valid.reshape(128, 16).astype(BF16)

    rowidx = (idx * BS + bb).astype(np.int32).reshape(128, NT)
    aug = np.ascontiguousarray(out_cat_logits[:, bsl]).reshape(R, K * C)
    return {"aug": aug.astype(FP8), "bigc": bigc, "rowidx": rowidx}


def _make_consts():
    pmat1 = np.zeros((K * K, NP_), np.float32)
    for p in range(NP_):
        for k in range(K):
            pmat1[k * K + PERMS[p, k], p] = 1.0
    pmat = np.zeros((128, 8 * NP_), np.float32)
    for tblk in range(8):
        pmat[tblk * 16:(tblk + 1) * 16, tblk * NP_:(tblk + 1) * NP_] = pmat1
    return {"pmat": pmat.astype(BF16), "ident": np.eye(128, dtype=BF16)}


def _build(nc, bass, mybir, tile):
    AP = bass.AP
    dt = mybir.dt
    Alu = mybir.AluOpType
    Act = mybir.ActivationFunctionType

    aug = nc.dram_tensor("aug", [R, K * C], dt.float8e4, kind="ExternalInput")
    bigc = nc.dram_tensor("bigc", [128, W_BIGC], dt.bfloat16,
                          kind="ExternalInput")
    rowidx = nc.dram_tensor("rowidx", [128, NT], dt.int32,
                            kind="ExternalInput")
    out = nc.dram_tensor("out", [1, 2], dt.float32, kind="ExternalOutput")

    with tile.TileContext(nc) as tc:
        with (
            tc.tile_pool(name="main", bufs=1) as pool,
            tc.tile_pool(name="psum", bufs=1, space="PSUM") as ppool,
        ):
            # rowidx first (gates the gathers); issued from gpsimd so the
            # indirects on the same queue start as early as possible.
            ri = pool.tile([128, NT], dt.int32)
            nc.gpsimd.dma_start(ri[:], rowidx.ap())
            cb = pool.tile([128, W_BIGC], dt.bfloat16)
            nc.sync.dma_start(cb[:], bigc.ap())

            picked_v = cb[:, O_PICK:O_PICK + 256]
            pmat_v = cb[:, O_PMAT:O_PMAT + 192]
            ident_v = cb[:, O_ID:O_ID + 128]
            valid_v = cb[:, O_VAL:O_VAL + 16]

            # ---- indirect row gathers, chunked for DMA/compute overlap
            G = pool.tile([128, NT * K * C], dt.float8e4)
            for st, w in GATHER_CHUNKS:
                nc.gpsimd.indirect_dma_start(
                    out=G[:, st * 1024:(st + w) * 1024], out_offset=None,
                    in_=aug.ap(),
                    in_offset=bass.IndirectOffsetOnAxis(ap=ri[:, st:st + w],
                                                        axis=0))

            # ---- cost16[n, k*4+t2] = |ot_k - dt_t2| + |oa_k - a_t2| - pick
            # (runs on DVE while the gathers stream)
            def rv(field, kstep, t2step):
                a = cb[:, O_REC + field:O_REC + field + 1]
                return AP(a.tensor, a.offset,
                          [list(a.ap[0]), [16, NT], [kstep, K], [t2step, K]])

            d1 = pool.tile([128, NT * K * K], dt.bfloat16)
            d13 = d1[:].rearrange("p (t a b) -> p t a b", a=K, b=K)
            nc.vector.tensor_tensor(out=d13, in0=rv(F_OT, 1, 0),
                                    in1=rv(F_DT, 0, 1), op=Alu.subtract)
            nc.vector.scalar_tensor_tensor(out=d1[:], in0=d1[:], scalar=-1.0,
                                           in1=d1[:], op0=Alu.mult,
                                           op1=Alu.max)
            d2 = pool.tile([128, NT * K * K], dt.bfloat16)
            d23 = d2[:].rearrange("p (t a b) -> p t a b", a=K, b=K)
            nc.vector.tensor_tensor(out=d23, in0=rv(F_OA, 1, 0),
                                    in1=rv(F_A, 0, 1), op=Alu.subtract)
            nc.vector.scalar_tensor_tensor(out=d2[:], in0=d2[:], scalar=-1.0,
                                           in1=d2[:], op0=Alu.mult,
                                           op1=Alu.max)
            cost16 = pool.tile([128, NT * K * K], dt.bfloat16)
            nc.vector.tensor_tensor(out=cost16[:], in0=d1[:], in1=d2[:],
                                    op=Alu.add)
            nc.vector.tensor_tensor(out=cost16[:], in0=cost16[:],
                                    in1=picked_v, op=Alu.subtract)

            # ---- 24-perm totals per half on PE, segmented min on DVE
            mint = pool.tile([128, NT], dt.float32)
            for h in range(2):
                pT = ppool.tile([128, 128], dt.bfloat16, tag=f"pT{h}")
                nc.tensor.transpose(out=pT[:],
                                    in_=cost16[:, h * 128:(h + 1) * 128],
                                    identity=ident_v)
                cT = pool.tile([128, 128], dt.bfloat16, tag=f"cT{h}")
                nc.vector.tensor_copy(out=cT[:], in_=pT[:])
                ptot = ppool.tile([128, 8 * NP_], dt.float32, tag=f"ptot{h}")
                nc.tensor.matmul(out=ptot[:], lhsT=cT[:], rhs=pmat_v,
                                 start=True, stop=True)
                nc.vector.tensor_reduce(
                    out=mint[:, h * 8:(h + 1) * 8],
                    in_=ptot[:].rearrange("p (t q) -> p t q", q=NP_),
                    axis=mybir.AxisListType.X, op=Alu.min)

            validf = pool.tile([128, NT], dt.float32)
            nc.vector.tensor_copy(out=validf[:], in_=valid_v)
            pair = pool.tile([128, 2], dt.float32)
            nc.vector.tensor_reduce(out=pair[:, 1:2], in_=validf[:],
                                    axis=mybir.AxisListType.X, op=Alu.add)

            # ---- exp (fp8 in, bf16 out), 2048-wide
            E = pool.tile([128, NT * K * C], dt.bfloat16)
            for st, w in EXP_CHUNKS:
                nc.scalar.activation(out=E[:, st * 1024:(st + w) * 1024],
                                     in_=G[:, st * 1024:(st + w) * 1024],
                                     func=Act.Exp)

            # ---- softmax denominators: per 4-tile group, a tree of
            # 2x-mode bf16 adds collapses (16 segs x 256) -> (16 x 16),
            # then one f32 tensor_reduce finishes the 16 segment sums.
            s4 = pool.tile([128, NT * K], dt.float32)
            tree = []
            for w in (128, 64, 32, 16):
                trtile = pool.tile([128, 16 * w], dt.bfloat16, tag=f"tr{w}",
                                   name=f"tr{w}")
                tree.append(trtile)

            def ev(base, nseg, segstep, width, off):
                a = E[:, base:base + 1]
                return AP(a.tensor, a.offset + off,
                          [list(a.ap[0]), [segstep, nseg], [1, width]])

            def tv(tl, nseg, segstride, width, off):
                a = tl[:, 0:1]
                return AP(a.tensor, a.offset + off,
                          [list(a.ap[0]), [segstride, nseg], [1, width]])

            for g in range(4):
                base = g * 4096
                nc.vector.tensor_tensor(out=tv(tree[0], 16, 128, 128, 0),
                                        in0=ev(base, 16, 256, 128, 0),
                                        in1=ev(base, 16, 256, 128, 128),
                                        op=Alu.add)
                for s in range(3):
                    w = 64 >> s
                    nc.vector.tensor_tensor(out=tv(tree[s + 1], 16, w, w, 0),
                                            in0=tv(tree[s], 16, 2 * w, w, 0),
                                            in1=tv(tree[s], 16, 2 * w, w, w),
                                            op=Alu.add)
                nc.vector.tensor_reduce(
                    out=s4[:, g * 16:(g + 1) * 16],
                    in_=tree[3][:].rearrange("p (s c) -> p s c", c=16),
                    axis=mybir.AxisListType.X, op=Alu.add)

            # ---- lse and final masked reduction
            lse = pool.tile([128, NT * K], dt.float32)
            nc.scalar.activation(out=lse[:], in_=s4[:], func=Act.Ln)
            slse = pool.tile([128, NT], dt.float32)
            nc.vector.tensor_reduce(
                out=slse[:], in_=lse[:].rearrange("p (t k) -> p t k", k=K),
                axis=mybir.AxisListType.X, op=Alu.add)
            item = pool.tile([128, NT], dt.float32)
            nc.vector.tensor_tensor(out=item[:], in0=mint[:], in1=slse[:],
                                    op=Alu.add)
            nc.vector.tensor_tensor(out=item[:], in0=item[:], in1=validf[:],
                                    op=Alu.mult)
            nc.vector.tensor_reduce(out=pair[:, 0:1], in_=item[:],
                                    axis=mybir.AxisListType.X, op=Alu.add)
            sb = pool.tile([1, 2], dt.float32)
            nc.gpsimd.tensor_reduce(out=sb[:], in_=pair[:],
                                    axis=mybir.AxisListType.C, op=Alu.add)
            nc.gpsimd.dma_start(out.ap(), sb[:])
    return nc


NCORES = 8
_COMPILED = {}


def _get_compiled():
    if "nc" not in _COMPILED:
        import concourse.bacc as bacc
        import concourse.bass as bass
        import concourse.mybir as mybir
        import concourse.tile as tile
        nc = bacc.Bacc("TRN2", target_bir_lowering=False, debug=False,
                       num_devices=NCORES)
        _build(nc, bass, mybir, tile)
        nc.compile()
        _COMPILED["nc"] = nc
    return _COMPILED["nc"]


def kernel(time, amount, out_time, out_amount, out_cat_logits, cat, lengths,
           indices):
    from concourse.bass_utils import run_bass_kernel_spmd

    time = np.asarray(time, dtype=np.float32)
    amount = np.asarray(amount, dtype=np.float32)
    out_time = np.asarray(out_time, dtype=np.float32)
    out_amount = np.asarray(out_amount, dtype=np.float32)
    out_cat_logits = np.asarray(out_cat_logits, dtype=np.float32)
    cat = np.asarray(cat, dtype=np.int32)
    lengths = np.asarray(lengths, dtype=np.int32)
    indices = np.asarray(indices, dtype=np.int32)

    nc = _get_compiled()
    consts = _make_consts()
    in_maps = [
        _host_prep(c, time, amount, out_time, out_amount, out_cat_logits, cat,
                   lengths, indices, consts)
        for c in range(NCORES)
    ]
    res = run_bass_kernel_spmd(nc, in_maps, core_ids=list(range(NCORES)))
    ls = sum(float(res.results[c]["out"][0, 0]) for c in range(NCORES))
    cn = sum(float(res.results[c]["out"][0, 1]) for c in range(NCORES))
    return np.float32(ls / (cn * K))


# revision 8
# speedup vs baseline: 1.2017x; 1.2017x over previous
"""DeTPP loss kernel for 8 TRN2 NeuronCores (batch-parallel SPMD Bass/Tile).

Strategy: shard along batch B (8 per core). Host prep does index plumbing
on tiny tensors (row ids, the assignment cost matrix built from the K*T
picked-category logits and L1 terms: ~0.3MB/core vs the 32MB logits
table). The memory-heavy work stays on device, per core:
  - one bf16 logits table [R, K*C] in DRAM; the 2048 needed rows (2KB
    each, 4MB total) are fetched with 8 chunked indirect row-gather DMAs
    (one SWDGE descriptor per row, 16 HW DMA engines in parallel),
  - 24-permutation assignment totals via PE (transpose + block-diagonal
    0/1 matmul per half) and a DVE segmented min = exact Hungarian
    optimum for K=4 -> per-item min cost, DMA'd out mid-stream,
  - ACT exp in wide instructions pipelined behind the gather chunks;
    per-(n,k) softmax denominators collapsed 256->16 by a tree of
    2x-mode bf16 tensor_tensor adds on DVE (tensor_reduce has no DVE
    fast mode), partial sums DMA'd out per group as they complete.
Host finishes the scalar epilogue on the shipped partials (16-way sum,
ln, masked mean) - ~1M flops vs the device's ~6M heavy ops on 4MB.
"""
import sys

sys.path.insert(0, '/opt/trn_rl_repo')

import itertools
import numpy as np
import ml_dtypes

BF16 = ml_dtypes.bfloat16

L, B, I, K, C = 1024, 64, 256, 4, 256
BS = B // 8            # batch per core
R = L * BS             # rows per core (8192), row id r = l*BS + b
N = I * BS             # gathered items per core (2048)
NT = N // 128          # 16 n-tiles; item n = p*NT + t  (p = partition)
PERMS = np.array(list(itertools.permutations(range(K))), dtype=np.int32)
NP_ = PERMS.shape[0]   # 24

# bigc packed-constant column layout (all bf16)
O_COST, O_PMAT, O_ID = 0, 256, 448
W_BIGC = 576

# (tile start, width): gather chunks == exp chunks, aligned
CHUNKS = [(0, 1), (1, 1), (2, 2), (4, 4), (8, 4), (12, 2), (14, 1), (15, 1)]
GROUPS = [(0, 4), (4, 4), (8, 4), (12, 2), (14, 2)]   # tree-sum groups


def _host_prep(core, time, amount, out_time, out_amount, out_cat_logits, cat,
               lengths, indices, consts):
    bsl = slice(core * BS, (core + 1) * BS)
    idx = indices[:, bsl].astype(np.int64)                    # (I, BS)
    bb = np.broadcast_to(np.arange(BS)[None, :], idx.shape)   # (I, BS)
    pos = (idx[:, :, None] + 1 + np.arange(K)[None, None, :]) % L  # (I,BS,K)
    bb3 = np.broadcast_to(bb[:, :, None], pos.shape)

    tloc = time[:, bsl]
    dt = tloc[pos, bb3] - tloc[idx, bb][:, :, None]           # (I, BS, K)
    aw = amount[:, bsl][pos, bb3]                             # (I, BS, K)
    cw = cat[:, bsl][pos, bb3].astype(np.int64)               # (I, BS, K)
    ot = out_time[:, bsl][idx, bb]                            # (I, BS, K)
    oa = out_amount[:, bsl][idx, bb]                          # (I, BS, K)
    ocl = out_cat_logits[:, bsl]                              # (L, BS, K, C)
    kk = np.arange(K)[None, None, :, None]
    picked = ocl[idx[:, :, None, None], bb[:, :, None, None], kk,
                 cw[:, :, None, :]]                           # (I, BS, K, T)
    # cost[n, k, t] = |ot_k - dt_t| + |oa_k - a_t| - picked_logit[k, t];
    # the lse part of the CE is added after the assignment min (it is
    # constant across the permutation search).
    cost = (np.abs(ot[:, :, :, None] - dt[:, :, None, :])
            + np.abs(oa[:, :, :, None] - aw[:, :, None, :])
            - picked)                                          # (I, BS, K, T)
    valid = (idx + K < lengths[bsl].astype(np.int64)[None, :])

    bigc = np.zeros((128, W_BIGC), BF16)
    bigc[:, O_COST:O_COST + 256] = cost.reshape(128, 256).astype(BF16)
    bigc[:, O_PMAT:O_PMAT + 192] = consts["pmat"]
    bigc[:, O_ID:O_ID + 128] = consts["ident"]

    rowidx = (idx * BS + bb).astype(np.int32).reshape(128, NT)
    aug = np.ascontiguousarray(out_cat_logits[:, bsl]).reshape(R, K * C)
    return ({"aug": aug.astype(BF16), "bigc": bigc, "rowidx": rowidx},
            valid.reshape(128, NT))


def _make_consts():
    pmat1 = np.zeros((K * K, NP_), np.float32)
    for p in range(NP_):
        for k in range(K):
            pmat1[k * K + PERMS[p, k], p] = 1.0
    pmat = np.zeros((128, 8 * NP_), np.float32)
    for tblk in range(8):
        pmat[tblk * 16:(tblk + 1) * 16, tblk * NP_:(tblk + 1) * NP_] = pmat1
    return {"pmat": pmat.astype(BF16), "ident": np.eye(128, dtype=BF16)}


def _build(nc, bass, mybir, tile):
    AP = bass.AP
    dt = mybir.dt
    Alu = mybir.AluOpType
    Act = mybir.ActivationFunctionType

    aug = nc.dram_tensor("aug", [R, K * C], dt.bfloat16, kind="ExternalInput")
    bigc = nc.dram_tensor("bigc", [128, W_BIGC], dt.bfloat16,
                          kind="ExternalInput")
    rowidx = nc.dram_tensor("rowidx", [128, NT], dt.int32,
                            kind="ExternalInput")
    mint_o = nc.dram_tensor("mint_o", [128, NT], dt.float32,
                            kind="ExternalOutput")
    s16_o = nc.dram_tensor("s16_o", [128, NT * K * 16], dt.bfloat16,
                           kind="ExternalOutput")

    with tile.TileContext(nc) as tc:
        with (
            tc.tile_pool(name="main", bufs=1) as pool,
            tc.tile_pool(name="psum", bufs=1, space="PSUM") as ppool,
        ):
            # rowidx first: it gates the whole gather stream
            ri = pool.tile([128, NT], dt.int32)
            nc.sync.dma_start(ri[:], rowidx.ap())
            cb = pool.tile([128, W_BIGC], dt.bfloat16)
            nc.sync.dma_start(cb[:], bigc.ap())

            pmat_v = cb[:, O_PMAT:O_PMAT + 192]
            ident_v = cb[:, O_ID:O_ID + 128]

            # ---- indirect row gathers, chunked for DMA/compute overlap
            G = pool.tile([128, NT * K * C], dt.bfloat16)
            for st, w in CHUNKS:
                nc.gpsimd.indirect_dma_start(
                    out=G[:, st * 1024:(st + w) * 1024], out_offset=None,
                    in_=aug.ap(),
                    in_offset=bass.IndirectOffsetOnAxis(ap=ri[:, st:st + w],
                                                        axis=0))

            # ---- 24-perm totals per half on PE, segmented min on DVE
            # (runs while the gathers stream; cost matrix is host-built)
            mint = pool.tile([128, NT], dt.float32)
            for h in range(2):
                pT = ppool.tile([128, 128], dt.bfloat16, tag=f"pT{h}")
                nc.tensor.transpose(out=pT[:],
                                    in_=cb[:, O_COST + h * 128:
                                           O_COST + (h + 1) * 128],
                                    identity=ident_v)
                cT = pool.tile([128, 128], dt.bfloat16, tag=f"cT{h}")
                nc.vector.tensor_copy(out=cT[:], in_=pT[:])
                ptot = ppool.tile([128, 8 * NP_], dt.float32, tag=f"ptot{h}")
                nc.tensor.matmul(out=ptot[:], lhsT=cT[:], rhs=pmat_v,
                                 start=True, stop=True)
                nc.vector.tensor_reduce(
                    out=mint[:, h * 8:(h + 1) * 8],
                    in_=ptot[:].rearrange("p (t q) -> p t q", q=NP_),
                    axis=mybir.AxisListType.X, op=Alu.min)
            nc.sync.dma_start(mint_o.ap(), mint[:])

            # ---- exp (bf16), chunk-aligned with the gathers
            E = pool.tile([128, NT * K * C], dt.bfloat16)
            for st, w in CHUNKS:
                nc.scalar.activation(out=E[:, st * 1024:(st + w) * 1024],
                                     in_=G[:, st * 1024:(st + w) * 1024],
                                     func=Act.Exp)

            # ---- softmax denominators: per group, a tree of 2x-mode bf16
            # adds collapses (segs x 256) -> (segs x 16); host sums the 16.
            s16 = pool.tile([128, NT * K * 16], dt.bfloat16)
            tree = []
            for w in (128, 64, 32):
                trtile = pool.tile([128, 4 * K * w], dt.bfloat16,
                                   tag=f"tr{w}", name=f"tr{w}")
                tree.append(trtile)

            def ev(nseg, segstep, width, off):
                a = E[:, 0:1]
                return AP(a.tensor, a.offset + off,
                          [list(a.ap[0]), [segstep, nseg], [1, width]])

            def tv(tl, nseg, segstride, width, off):
                a = tl[:, 0:1]
                return AP(a.tensor, a.offset + off,
                          [list(a.ap[0]), [segstride, nseg], [1, width]])

            def sv(nseg, width, off):
                a = s16[:, 0:1]
                return AP(a.tensor, a.offset + off,
                          [list(a.ap[0]), [16, nseg], [1, width]])

            for gt, gw in GROUPS:
                ns = gw * K
                nc.vector.tensor_tensor(
                    out=tv(tree[0], ns, 128, 128, 0),
                    in0=ev(ns, 256, 128, gt * 1024),
                    in1=ev(ns, 256, 128, gt * 1024 + 128), op=Alu.add)
                nc.vector.tensor_tensor(
                    out=tv(tree[1], ns, 64, 64, 0),
                    in0=tv(tree[0], ns, 128, 64, 0),
                    in1=tv(tree[0], ns, 128, 64, 64), op=Alu.add)
                nc.vector.tensor_tensor(
                    out=tv(tree[2], ns, 32, 32, 0),
                    in0=tv(tree[1], ns, 64, 32, 0),
                    in1=tv(tree[1], ns, 64, 32, 32), op=Alu.add)
                nc.vector.tensor_tensor(
                    out=sv(ns, 16, gt * K * 16),
                    in0=tv(tree[2], ns, 32, 16, 0),
                    in1=tv(tree[2], ns, 32, 16, 16), op=Alu.add)
                nc.sync.dma_start(
                    s16_o.ap()[:, gt * K * 16:(gt + gw) * K * 16],
                    s16[:, gt * K * 16:(gt + gw) * K * 16])
    return nc


NCORES = 8
_COMPILED = {}


def _get_compiled():
    if "nc" not in _COMPILED:
        import concourse.bacc as bacc
        import concourse.bass as bass
        import concourse.mybir as mybir
        import concourse.tile as tile
        nc = bacc.Bacc("TRN2", target_bir_lowering=False, debug=False,
                       num_devices=NCORES)
        _build(nc, bass, mybir, tile)
        nc.compile()
        _COMPILED["nc"] = nc
    return _COMPILED["nc"]


def kernel(time, amount, out_time, out_amount, out_cat_logits, cat, lengths,
           indices):
    from concourse.bass_utils import run_bass_kernel_spmd

    time = np.asarray(time, dtype=np.float32)
    amount = np.asarray(amount, dtype=np.float32)
    out_time = np.asarray(out_time, dtype=np.float32)
    out_amount = np.asarray(out_amount, dtype=np.float32)
    out_cat_logits = np.asarray(out_cat_logits, dtype=np.float32)
    cat = np.asarray(cat, dtype=np.int32)
    lengths = np.asarray(lengths, dtype=np.int32)
    indices = np.asarray(indices, dtype=np.int32)

    nc = _get_compiled()
    consts = _make_consts()
    in_maps, valids = [], []
    for c in range(NCORES):
        m, v = _host_prep(c, time, amount, out_time, out_amount,
                          out_cat_logits, cat, lengths, indices, consts)
        in_maps.append(m)
        valids.append(v)
    res = run_bass_kernel_spmd(nc, in_maps, core_ids=list(range(NCORES)))
    ls, cn = 0.0, 0.0
    for c in range(NCORES):
        mint = res.results[c]["mint_o"]                       # (128, NT) f32
        s16 = res.results[c]["s16_o"].astype(np.float32)      # (128, 1024)
        s4 = s16.reshape(128, NT * K, 16).sum(-1)             # (128, 64)
        slse = np.log(s4).reshape(128, NT, K).sum(-1)         # (128, NT)
        v = valids[c]
        ls += float(((mint + slse) * v).sum())
        cn += float(v.sum())
    return np.float32(ls / (cn * K))


# revision 10
# speedup vs baseline: 1.2443x; 1.0355x over previous
"""DeTPP loss kernel for 8 TRN2 NeuronCores (batch-parallel SPMD Bass/Tile).

Strategy: shard along batch B (8 per core). Host prep does index plumbing
on tiny tensors (row ids, the assignment cost matrix built from the K*T
picked-category logits and L1 terms: ~0.3MB/core vs the 32MB logits
table). The memory-heavy work stays on device, per core:
  - one bf16 logits table [R, K*C] in DRAM; the 2048 needed rows (2KB
    each, 4MB total) are fetched with 8 chunked indirect row-gather DMAs
    (one SWDGE descriptor per row, 16 HW DMA engines in parallel),
  - 24-permutation assignment totals via PE (transpose + block-diagonal
    0/1 matmul per half) and a DVE segmented min = exact Hungarian
    optimum for K=4 -> per-item min cost, DMA'd out mid-stream,
  - ACT exp in wide instructions pipelined behind the gather chunks;
    per-(n,k) softmax denominators collapsed 256->16 by a tree of
    2x-mode bf16 tensor_tensor adds on DVE (tensor_reduce has no DVE
    fast mode), partial sums DMA'd out per group as they complete.
Host finishes the scalar epilogue on the shipped partials (16-way sum,
ln, masked mean) - ~1M flops vs the device's ~6M heavy ops on 4MB.
"""
import sys

sys.path.insert(0, '/opt/trn_rl_repo')

import itertools
import numpy as np
import ml_dtypes

BF16 = ml_dtypes.bfloat16

L, B, I, K, C = 1024, 64, 256, 4, 256
BS = B // 8            # batch per core
R = L * BS             # rows per core (8192), row id r = l*BS + b
N = I * BS             # gathered items per core (2048)
NT = N // 128          # 16 n-tiles; item n = p*NT + t  (p = partition)
PERMS = np.array(list(itertools.permutations(range(K))), dtype=np.int32)
NP_ = PERMS.shape[0]   # 24

# bigc packed-constant column layout (all bf16)
O_COST, O_PMAT, O_ID = 0, 256, 448
W_BIGC = 576

# (tile start, width): gather chunks == exp chunks, aligned
CHUNKS = [(t, 2) for t in range(0, NT, 2)]
GROUPS = [(0, 4), (4, 4), (8, 4), (12, 2), (14, 2)]   # tree-sum groups

# slot (p, t) <-> sorted-row rank, so each gather chunk's descriptors hit
# strictly ascending table rows (DRAM locality); chunk (st, w) enumerates
# its offset AP p-major, rank = st*128 + p*w + (t - st).
SLOTMAP = np.empty((128, NT), np.int64)
for _st, _w in CHUNKS:
    for _p in range(128):
        SLOTMAP[_p, _st:_st + _w] = _st * 128 + _p * _w + np.arange(_w)


def _host_prep(core, time, amount, out_time, out_amount, out_cat_logits, cat,
               lengths, indices, consts):
    bsl = slice(core * BS, (core + 1) * BS)
    idx = indices[:, bsl].astype(np.int64)                    # (I, BS)
    bb = np.broadcast_to(np.arange(BS)[None, :], idx.shape)   # (I, BS)
    pos = (idx[:, :, None] + 1 + np.arange(K)[None, None, :]) % L  # (I,BS,K)
    bb3 = np.broadcast_to(bb[:, :, None], pos.shape)

    tloc = time[:, bsl]
    dt = tloc[pos, bb3] - tloc[idx, bb][:, :, None]           # (I, BS, K)
    aw = amount[:, bsl][pos, bb3]                             # (I, BS, K)
    cw = cat[:, bsl][pos, bb3].astype(np.int64)               # (I, BS, K)
    ot = out_time[:, bsl][idx, bb]                            # (I, BS, K)
    oa = out_amount[:, bsl][idx, bb]                          # (I, BS, K)
    ocl = out_cat_logits[:, bsl]                              # (L, BS, K, C)
    kk = np.arange(K)[None, None, :, None]
    picked = ocl[idx[:, :, None, None], bb[:, :, None, None], kk,
                 cw[:, :, None, :]]                           # (I, BS, K, T)
    # cost[n, k, t] = |ot_k - dt_t| + |oa_k - a_t| - picked_logit[k, t];
    # the lse part of the CE is added after the assignment min (it is
    # constant across the permutation search).
    cost = (np.abs(ot[:, :, :, None] - dt[:, :, None, :])
            + np.abs(oa[:, :, :, None] - aw[:, :, None, :])
            - picked)                                          # (I, BS, K, T)
    valid = (idx + K < lengths[bsl].astype(np.int64)[None, :])

    # sort items by table row and place rank SLOTMAP[p,t] at slot (p,t):
    # the gather descriptors then walk the table in ascending address order
    rows = (idx * BS + bb).reshape(N)
    order = np.argsort(rows, kind="stable")[SLOTMAP]          # (128, NT)
    rowidx = rows[order].astype(np.int32)
    cost_s = cost.reshape(N, K * K)[order]                    # (128, NT, 16)
    valid_s = valid.reshape(N)[order]                         # (128, NT)

    bigc = np.zeros((128, W_BIGC), BF16)
    bigc[:, O_COST:O_COST + 256] = cost_s.reshape(128, 256).astype(BF16)
    bigc[:, O_PMAT:O_PMAT + 192] = consts["pmat"]
    bigc[:, O_ID:O_ID + 128] = consts["ident"]

    aug = np.ascontiguousarray(out_cat_logits[:, bsl]).reshape(R, K * C)
    return ({"aug": aug.astype(BF16), "bigc": bigc, "rowidx": rowidx},
            valid_s)


def _make_consts():
    pmat1 = np.zeros((K * K, NP_), np.float32)
    for p in range(NP_):
        for k in range(K):
            pmat1[k * K + PERMS[p, k], p] = 1.0
    pmat = np.zeros((128, 8 * NP_), np.float32)
    for tblk in range(8):
        pmat[tblk * 16:(tblk + 1) * 16, tblk * NP_:(tblk + 1) * NP_] = pmat1
    return {"pmat": pmat.astype(BF16), "ident": np.eye(128, dtype=BF16)}


def _build(nc, bass, mybir, tile):
    AP = bass.AP
    dt = mybir.dt
    Alu = mybir.AluOpType
    Act = mybir.ActivationFunctionType

    aug = nc.dram_tensor("aug", [R, K * C], dt.bfloat16, kind="ExternalInput")
    bigc = nc.dram_tensor("bigc", [128, W_BIGC], dt.bfloat16,
                          kind="ExternalInput")
    rowidx = nc.dram_tensor("rowidx", [128, NT], dt.int32,
                            kind="ExternalInput")
    mint_o = nc.dram_tensor("mint_o", [128, NT], dt.float32,
                            kind="ExternalOutput")
    s16_o = nc.dram_tensor("s16_o", [128, NT * K * 16], dt.bfloat16,
                           kind="ExternalOutput")

    with tile.TileContext(nc) as tc:
        with (
            tc.tile_pool(name="main", bufs=1) as pool,
            tc.tile_pool(name="psum", bufs=1, space="PSUM") as ppool,
        ):
            # rowidx first: it gates the whole gather stream
            ri = pool.tile([128, NT], dt.int32)
            nc.sync.dma_start(ri[:], rowidx.ap())
            cb = pool.tile([128, W_BIGC], dt.bfloat16)
            nc.sync.dma_start(cb[:], bigc.ap())

            pmat_v = cb[:, O_PMAT:O_PMAT + 192]
            ident_v = cb[:, O_ID:O_ID + 128]

            # ---- indirect row gathers, chunked for DMA/compute overlap
            G = pool.tile([128, NT * K * C], dt.bfloat16)
            for st, w in CHUNKS:
                nc.gpsimd.indirect_dma_start(
                    out=G[:, st * 1024:(st + w) * 1024], out_offset=None,
                    in_=aug.ap(),
                    in_offset=bass.IndirectOffsetOnAxis(ap=ri[:, st:st + w],
                                                        axis=0))

            # ---- 24-perm totals per half on PE, segmented min on DVE
            # (runs while the gathers stream; cost matrix is host-built)
            mint = pool.tile([128, NT], dt.float32)
            for h in range(2):
                pT = ppool.tile([128, 128], dt.bfloat16, tag=f"pT{h}")
                nc.tensor.transpose(out=pT[:],
                                    in_=cb[:, O_COST + h * 128:
                                           O_COST + (h + 1) * 128],
                                    identity=ident_v)
                cT = pool.tile([128, 128], dt.bfloat16, tag=f"cT{h}")
                nc.vector.tensor_copy(out=cT[:], in_=pT[:])
                ptot = ppool.tile([128, 8 * NP_], dt.float32, tag=f"ptot{h}")
                nc.tensor.matmul(out=ptot[:], lhsT=cT[:], rhs=pmat_v,
                                 start=True, stop=True)
                nc.vector.tensor_reduce(
                    out=mint[:, h * 8:(h + 1) * 8],
                    in_=ptot[:].rearrange("p (t q) -> p t q", q=NP_),
                    axis=mybir.AxisListType.X, op=Alu.min)
            nc.sync.dma_start(mint_o.ap(), mint[:])

            # ---- exp (bf16), chunk-aligned with the gathers
            E = pool.tile([128, NT * K * C], dt.bfloat16)
            for st, w in CHUNKS:
                nc.scalar.activation(out=E[:, st * 1024:(st + w) * 1024],
                                     in_=G[:, st * 1024:(st + w) * 1024],
                                     func=Act.Exp)

            # ---- softmax denominators: per group, a tree of 2x-mode bf16
            # adds collapses (segs x 256) -> (segs x 16); host sums the 16.
            s16 = pool.tile([128, NT * K * 16], dt.bfloat16)
            tree = []
            for w in (128, 64, 32):
                trtile = pool.tile([128, 4 * K * w], dt.bfloat16,
                                   tag=f"tr{w}", name=f"tr{w}")
                tree.append(trtile)

            def ev(nseg, segstep, width, off):
                a = E[:, 0:1]
                return AP(a.tensor, a.offset + off,
                          [list(a.ap[0]), [segstep, nseg], [1, width]])

            def tv(tl, nseg, segstride, width, off):
                a = tl[:, 0:1]
                return AP(a.tensor, a.offset + off,
                          [list(a.ap[0]), [segstride, nseg], [1, width]])

            def sv(nseg, width, off):
                a = s16[:, 0:1]
                return AP(a.tensor, a.offset + off,
                          [list(a.ap[0]), [16, nseg], [1, width]])

            for gt, gw in GROUPS:
                ns = gw * K
                nc.vector.tensor_tensor(
                    out=tv(tree[0], ns, 128, 128, 0),
                    in0=ev(ns, 256, 128, gt * 1024),
                    in1=ev(ns, 256, 128, gt * 1024 + 128), op=Alu.add)
                nc.vector.tensor_tensor(
                    out=tv(tree[1], ns, 64, 64, 0),
                    in0=tv(tree[0], ns, 128, 64, 0),
                    in1=tv(tree[0], ns, 128, 64, 64), op=Alu.add)
                nc.vector.tensor_tensor(
                    out=tv(tree[2], ns, 32, 32, 0),
                    in0=tv(tree[1], ns, 64, 32, 0),
                    in1=tv(tree[1], ns, 64, 32, 32), op=Alu.add)
                nc.vector.tensor_tensor(
                    out=sv(ns, 16, gt * K * 16),
                    in0=tv(tree[2], ns, 32, 16, 0),
                    in1=tv(tree[2], ns, 32, 16, 16), op=Alu.add)
                nc.sync.dma_start(
                    s16_o.ap()[:, gt * K * 16:(gt + gw) * K * 16],
                    s16[:, gt * K * 16:(gt + gw) * K * 16])
    return nc


NCORES = 8
_COMPILED = {}


def _get_compiled():
    if "nc" not in _COMPILED:
        import concourse.bacc as bacc
        import concourse.bass as bass
        import concourse.mybir as mybir
        import concourse.tile as tile
        nc = bacc.Bacc("TRN2", target_bir_lowering=False, debug=False,
                       num_devices=NCORES)
        _build(nc, bass, mybir, tile)
        nc.compile()
        _COMPILED["nc"] = nc
    return _COMPILED["nc"]


def kernel(time, amount, out_time, out_amount, out_cat_logits, cat, lengths,
           indices):
    from concourse.bass_utils import run_bass_kernel_spmd

    time = np.asarray(time, dtype=np.float32)
    amount = np.asarray(amount, dtype=np.float32)
    out_time = np.asarray(out_time, dtype=np.float32)
    out_amount = np.asarray(out_amount, dtype=np.float32)
    out_cat_logits = np.asarray(out_cat_logits, dtype=np.float32)
    cat = np.asarray(cat, dtype=np.int32)
    lengths = np.asarray(lengths, dtype=np.int32)
    indices = np.asarray(indices, dtype=np.int32)

    nc = _get_compiled()
    consts = _make_consts()
    in_maps, valids = [], []
    for c in range(NCORES):
        m, v = _host_prep(c, time, amount, out_time, out_amount,
                          out_cat_logits, cat, lengths, indices, consts)
        in_maps.append(m)
        valids.append(v)
    res = run_bass_kernel_spmd(nc, in_maps, core_ids=list(range(NCORES)))
    ls, cn = 0.0, 0.0
    for c in range(NCORES):
        mint = res.results[c]["mint_o"]                       # (128, NT) f32
        s16 = res.results[c]["s16_o"].astype(np.float32)      # (128, 1024)
        s4 = s16.reshape(128, NT * K, 16).sum(-1)             # (128, 64)
        slse = np.log(s4).reshape(128, NT, K).sum(-1)         # (128, NT)
        v = valids[c]
        ls += float(((mint + slse) * v).sum())
        cn += float(v.sum())
    return np.float32(ls / (cn * K))
